# revision 37
# baseline (speedup 1.0000x reference)
"""Trainium2 Bass kernel for a plastic (Hebbian fast-weight) LSTM cell.

Shapes (hardcoded): B=32, D=512, H=1024, 8 NeuronCores.
Sharding: hidden-dim sharding. Core c owns the h-slice [c*128, (c+1)*128) of:
  - Hebb0/Hebb1 last axis, h1/c1 columns, weight_h/weight_x/bias gate-column
    slices, alpha columns.
x / h0 are replicated (tiny). This beats batch sharding because the 28MB of
replicated weights dominate per-core DMA traffic otherwise.

Math per core (hs = 128-wide h slice, transposed "T" = h-major layout):
  gatesT[c, b]  = sum_k Wh[k, c] h0[b, k] + sum_d Wx[d, c] x[b, d] + bias[c]
  plastic[b, h] = sum_i h0[b,i] alpha[i,h] Hebb0[b,i,h]  (PE contraction of
                  m = alpha*Hebb0 against h0 columns)
  tg = tanh(g_gate + plastic);  c1 = sig(f)*c0 + sig(i)*tg; h1 = sig(o)*tanh(c1)
  Hebb1 = clip(Hebb0 + eta * h0[:,j] * tg[:,k], -1, 1)
"""

import os
from contextlib import ExitStack

import numpy as np

# The Bass kernel executes through jax/PJRT on the axon-tunneled NeuronCores;
# make sure the axon platform is visible even if the caller pinned cpu.
_p = os.environ.get("JAX_PLATFORMS")
if not _p:
    os.environ["JAX_PLATFORMS"] = "axon,cpu"
elif "axon" not in _p:
    os.environ["JAX_PLATFORMS"] = "axon," + _p

import concourse.bacc as bacc
import concourse.bass as bass
import concourse.mybir as mybir
import concourse.tile as tile
from concourse import dve_ops as _dvo
from concourse.bass_utils import run_bass_kernel_spmd
from concourse.dve_spec import (
    C0 as _C0,
    One as _One,
    Spec as _Spec,
    Src0 as _Src0,
    Src1 as _Src1,
    Zero as _Zero,
    lower as _lower,
    maxx as _maxx,
    minn as _minn,
)
from concourse.dve_uop import DveOpSpec as _DveOpSpec


def _get_fused_update_op():
    """Fused DVE op: out = clip(in0 * s0 + in1, -1, 1).

    One pass produces the updated Hebbian trace chunk from the tanh-gate
    broadcast (in0), per-partition eta*h0 column (s0), and Hebb0 (in1) —
    replacing scalar_tensor_tensor + a separate dual-scalar clip."""
    name = "PLASTIC_UPD_ANT"
    for op in _dvo.OPS:
        if op.name == name:
            return op
    spec = _Spec(
        body=_minn(_maxx(_Src0 * _C0 + _Src1, _Zero - _One), _One),
        reference=lambda in0, in1, c0, c1, c2: np.clip(in0 * c0 + in1, -1.0, 1.0),
    )
    opcode = max(_dvo._SUB_OPCODE_FOR_NAME.values()) + 1
    shas = {}
    for ver in ("v3", "v4"):
        s = _DveOpSpec(name=name, opcode=opcode, uops=_lower(spec, ver=ver),
                       rd1_en=_dvo.has_src1(spec))
        shas[ver] = s.sha(ver)
    op = _dvo.DveOp(name, spec, subdim=False, uops_sha=shas)
    _dvo.OPS.append(op)
    _dvo._SUB_OPCODE_FOR_NAME[name] = opcode
    _dvo.CUSTOM_DVE_SPECS[name] = spec
    return op

B, D, H, NCORES = 32, 512, 1024, 8
HS = H // NCORES  # 128
P = 128
F32 = mybir.dt.float32
GRP = 8  # batch-group size for the tanh/transpose stage
ALU = mybir.AluOpType
AFT = mybir.ActivationFunctionType

KC_H = H // P  # 8 contraction chunks for weight_h / alpha / hebb rows
KC_X = D // P  # 4 contraction chunks for weight_x


BF16 = mybir.dt.bfloat16


def build_kernel(hb_bufs=20, upd_bufs=4, m_bufs=3, bf16_m=False):
    """Build the per-core Bass/Tile program. Same program runs SPMD on all
    8 cores with different data."""
    nc = bacc.Bacc("TRN2", target_bir_lowering=False, debug=False, num_devices=NCORES)

    ein = lambda name, shape: nc.dram_tensor(name, shape, F32, kind="ExternalInput").ap()
    eout = lambda name, shape: nc.dram_tensor(name, shape, F32, kind="ExternalOutput").ap()

    hebb = ein("hebb", [B, H, HS])
    wh = ein("wh", [H, 4 * HS])
    wx = ein("wx", [D, 4 * HS])
    biasd = ein("biasv", [1, 4 * HS])
    alphad = ein("alphav", [H, HS])
    h0t = ein("h0t", [H, B])
    xt = ein("xt", [D, B])
    c0n = ein("c0n", [B, HS])
    etad = ein("etav", [1, 1])
    identd = ein("ident", [P, P])

    h1n = eout("h1n", [B, HS])
    c1n = eout("c1n", [B, HS])
    hebb1 = eout("hebb1", [B, H, HS])

    # Row-major partition split: partition p holds rows 8p..8p+7, so each
    # partition's DMA segment is 4KB contiguous (128 descriptors per DMA).
    hebb_r = hebb.rearrange("b (p r) k -> b p r k", r=8)
    hebb1_r = hebb1.rearrange("b (p r) k -> b p r k", r=8)

    with tile.TileContext(nc) as tc, ExitStack() as ctx:
        const = ctx.enter_context(tc.tile_pool(name="const", bufs=1))
        hpool = ctx.enter_context(tc.tile_pool(name="hebbp", bufs=hb_bufs))
        mpool = ctx.enter_context(tc.tile_pool(name="mp", bufs=m_bufs))
        upool = ctx.enter_context(tc.tile_pool(name="up", bufs=upd_bufs))
        tgbp = ctx.enter_context(tc.tile_pool(name="tgbp", bufs=3))
        tgrp = ctx.enter_context(tc.tile_pool(name="tgrp", bufs=3))
        smallp = ctx.enter_context(tc.tile_pool(name="smallp", bufs=2))
        psA = ctx.enter_context(tc.tile_pool(name="psA", bufs=1, space="PSUM"))
        psB = ctx.enter_context(tc.tile_pool(name="psB", bufs=2, space="PSUM"))
        psC = ctx.enter_context(tc.tile_pool(name="psC", bufs=3, space="PSUM"))

        dma = nc.sync.dma_start  # HWDGE
        fused_upd = _get_fused_update_op()

        # ---- critical-path constants first: what the DVE pipeline needs ----
        alpha_sb = const.tile([P, KC_H * HS], F32)  # [128, 1024]
        dma(alpha_sb[:].rearrange("p (r k) -> p r k", r=8),
            alphad.rearrange("(p r) k -> p r k", r=8))
        h0t2_sb = const.tile([P, 8 * B], F32)  # [128, 256] i = 8p + r layout
        dma(h0t2_sb[:].rearrange("p (r b) -> p r b", r=8),
            h0t.rearrange("(p r) b -> p r b", r=8))
        eta_sb = const.tile([1, 1], F32)
        dma(eta_sb[:], etad)
        ident_sb = const.tile([P, P], F32)
        dma(ident_sb[:], identd)

        # Prefetch the first group of Hebb tiles ahead of the big weights so
        # the elementwise pipeline starts as soon as possible.
        hbs = {}
        for b in range(GRP):
            hb = hpool.tile([P, KC_H * HS], F32, tag="hb")
            dma(hb[:].rearrange("p (r k) -> p r k", r=8), hebb_r[b])
            hbs[b] = hb

        # ---- remaining constants ----
        wh_sb = const.tile([P, KC_H * 4 * HS], F32)  # [128, 4096]
        dma(wh_sb[:].rearrange("p (kc c) -> p kc c", kc=KC_H),
            wh.rearrange("(kc p) c -> p kc c", p=P))
        wx_sb = const.tile([P, KC_X * 4 * HS], F32)  # [128, 2048]
        dma(wx_sb[:].rearrange("p (dc c) -> p dc c", dc=KC_X),
            wx.rearrange("(dc p) c -> p dc c", p=P))
        h0t_sb = const.tile([P, KC_H * B], F32)  # [128, 256] gates layout
        dma(h0t_sb[:].rearrange("p (kc b) -> p kc b", kc=KC_H),
            h0t.rearrange("(kc p) b -> p kc b", p=P))
        xt_sb = const.tile([P, KC_X * B], F32)  # [128, 128]
        dma(xt_sb[:].rearrange("p (dc b) -> p dc b", dc=KC_X),
            xt.rearrange("(dc p) b -> p dc b", p=P))
        c0n_sb = const.tile([B, HS], F32)
        dma(c0n_sb[:], c0n)
        bias_sb = const.tile([1, 4 * HS], F32)
        dma(bias_sb[:], biasd)

        ones_sb = const.tile([1, P], F32)
        nc.vector.memset(ones_sb[:], 1.0)

        # eta broadcast to all 128 partitions: ones_col @ eta
        eta_ps = psC.tile([P, 1], F32, tag="small")
        nc.tensor.matmul(eta_ps[:], ones_sb[:, 0:P], eta_sb[:], start=True, stop=True)
        eta128_sb = const.tile([P, 1], F32)
        nc.scalar.copy(eta128_sb[:], eta_ps[:])

        # eh0t = eta * h0T in the i = 8p + r layout (per-partition scale)
        eh0t_sb = const.tile([P, 8 * B], F32)
        nc.scalar.mul(eh0t_sb[:], h0t2_sb[:], eta128_sb[:, 0:1])

        mdt = BF16 if bf16_m else F32
        if bf16_m:
            h0t2_mm = const.tile([P, 8 * B], BF16)
            nc.vector.tensor_copy(h0t2_mm[:], h0t2_sb[:])
        else:
            h0t2_mm = h0t2_sb

        # ---- gate matmuls (transposed: [c, b]) ----
        pg_fio = psA.tile([P, 3 * B], F32)  # f|i|o chunks, one bank
        pg_g = psA.tile([P, B], F32)

        for q in range(4):
            out = pg_g[:] if q == 3 else pg_fio[:, q * B:(q + 1) * B]
            for kc in range(KC_H):
                nc.tensor.matmul(
                    out, wh_sb[:, kc * 512 + q * HS: kc * 512 + (q + 1) * HS],
                    h0t_sb[:, kc * B:(kc + 1) * B],
                    start=(kc == 0), stop=False)
            for dc in range(KC_X):
                nc.tensor.matmul(
                    out, wx_sb[:, dc * 512 + q * HS: dc * 512 + (q + 1) * HS],
                    xt_sb[:, dc * B:(dc + 1) * B],
                    start=False, stop=False)
            nc.tensor.matmul(out, bias_sb[:, q * HS:(q + 1) * HS],
                             ones_sb[:, 0:B], start=False, stop=True)

        sig_sb = const.tile([P, 3 * B], F32)
        nc.scalar.activation(sig_sb[:], pg_fio[:], AFT.Sigmoid)

        tgnat_sb = const.tile([B, HS], F32)  # tanh-gate rows, natural [b, k]

        for b in range(B):
            if b in hbs:
                hb = hbs.pop(b)
            else:
                hb = hpool.tile([P, KC_H * HS], F32, tag="hb")  # [128, 1024]
                dma(hb[:].rearrange("p (r k) -> p r k", r=8), hebb_r[b])
            m = mpool.tile([P, KC_H * HS], mdt, tag="m")
            nc.vector.tensor_mul(m[:], hb[:], alpha_sb[:])
            # plastic[b] + g-gate as one PSUM row: h0 column stationary (P=1
            # ldweights), m chunks moving; then the g-gate column transposed
            # in (also P=1 ldweights) to finish g_g[b] + plastic[b].
            pl_row = psC.tile([1, HS], F32, tag="small")
            for r in range(8):
                nc.tensor.matmul(
                    pl_row[:],
                    h0t2_mm[:, r * B + b: r * B + b + 1],
                    m[:, r * HS:(r + 1) * HS],
                    start=(r == 0), stop=False)
            gcol_sb = tgrp.tile([P, 1], F32, tag="gcol")
            nc.scalar.copy(gcol_sb[:], pg_g[:, b:b + 1])
            nc.tensor.matmul(pl_row[:], gcol_sb[:], ident_sb[:],
                             start=False, stop=True, is_transpose=True)
            # tanh directly on the row -> tg[b] at partition 0
            tgrow = tgrp.tile([1, P], F32, tag="tgrow")
            nc.scalar.activation(tgrow[:], pl_row[:], AFT.Tanh)
            # stash the row for the c1/h1 tail (sb->sb DMA moves partitions)
            nc.scalar.dma_start(tgnat_sb[b:b + 1, :], tgrow[:])
            # broadcast row to 128 partitions
            tgb_ps = psB.tile([P, P], F32, tag="tgb")
            nc.tensor.matmul(tgb_ps[:], ones_sb[:, 0:P], tgrow[:],
                             start=True, stop=True)
            tgb_sb = tgbp.tile([P, P], F32, tag="tgbs")
            nc.scalar.copy(tgb_sb[:], tgb_ps[:])
            # upd = clip(Hebb0 + (eta*h0_col) * tg_bcast, -1, 1) fused
            upd = upool.tile([P, KC_H * HS], F32, tag="upd")
            for r in range(8):
                nc.vector._custom_dve(
                    fused_upd,
                    out=upd[:, r * HS:(r + 1) * HS],
                    in0=tgb_sb[:],
                    in1=hb[:, r * HS:(r + 1) * HS],
                    s0=eh0t_sb[:, r * B + b: r * B + b + 1])
            nc.scalar.dma_start(
                hebb1_r[b], upd[:].rearrange("p (r k) -> p r k", r=8))

        # ---- tail: c1, h1 in natural [b, k] orientation ----
        # PE transposes turn the sigmoid columns into natural rows (each at
        # partition base 0 — tensor ops need matching base partitions)
        signat = []
        for q in range(3):
            sp = psB.tile([B, P], F32, tag="tgb")
            nc.tensor.transpose(sp[:], sig_sb[:, q * B:(q + 1) * B], ident_sb[:])
            sn = const.tile([B, P], F32)
            nc.scalar.copy(sn[:], sp[:])
            signat.append(sn)
        tmp1 = const.tile([B, HS], F32)
        tmp2 = const.tile([B, HS], F32)
        nc.vector.tensor_mul(tmp1[:], signat[0][:], c0n_sb[:])
        nc.vector.tensor_mul(tmp2[:], signat[1][:], tgnat_sb[:])
        c1n_sb = const.tile([B, HS], F32)
        nc.vector.tensor_add(c1n_sb[:], tmp1[:], tmp2[:])
        dma(c1n, c1n_sb[:])
        th_sb = const.tile([B, HS], F32)
        nc.scalar.activation(th_sb[:], c1n_sb[:], AFT.Tanh)
        h1n_sb = const.tile([B, HS], F32)
        nc.vector.tensor_mul(h1n_sb[:], signat[2][:], th_sb[:])
        dma(h1n, h1n_sb[:])

    nc.compile()
    return nc


def shard_inputs(x, h0, c0, Hebb0, weight_h, weight_x, bias, alpha, eta):
    """Host-side layout prep + hidden-dim sharding -> per-core input maps."""
    f32 = np.float32
    h0t = np.ascontiguousarray(h0.T, dtype=f32)
    xt = np.ascontiguousarray(x.T, dtype=f32)
    ident = np.eye(P, dtype=f32)
    eta2 = np.asarray(eta, dtype=f32).reshape(1, 1)
    in_maps = []
    for c in range(NCORES):
        sl = slice(c * HS, (c + 1) * HS)
        wh_c = np.ascontiguousarray(
            np.concatenate([weight_h[:, g * H:][:, sl] for g in range(4)], axis=1),
            dtype=f32)
        wx_c = np.ascontiguousarray(
            np.concatenate([weight_x[:, g * H:][:, sl] for g in range(4)], axis=1),
            dtype=f32)
        bias_c = np.ascontiguousarray(
            np.concatenate([bias[g * H:][sl] for g in range(4)])[None, :], dtype=f32)
        in_maps.append({
            "hebb": np.ascontiguousarray(Hebb0[:, :, sl], dtype=f32),
            "wh": wh_c,
            "wx": wx_c,
            "biasv": bias_c,
            "alphav": np.ascontiguousarray(alpha[:, sl], dtype=f32),
            "h0t": h0t,
            "xt": xt,
            "c0n": np.ascontiguousarray(c0[:, sl], dtype=f32),
            "etav": eta2,
            "ident": ident,
        })
    return in_maps


def gather_outputs(results):
    h1 = np.empty((B, H), dtype=np.float32)
    c1 = np.empty((B, H), dtype=np.float32)
    Hebb1 = np.empty((B, H, H), dtype=np.float32)
    for c in range(NCORES):
        sl = slice(c * HS, (c + 1) * HS)
        r = results[c]
        h1[:, sl] = r["h1n"]
        c1[:, sl] = r["c1n"]
        Hebb1[:, :, sl] = r["hebb1"]
    return h1, c1, Hebb1


_NC_CACHE = {}


def kernel(x, h0, c0, Hebb0, weight_h, weight_x, bias, alpha, eta):
    if "nc" not in _NC_CACHE:
        _NC_CACHE["nc"] = build_kernel()
    nc = _NC_CACHE["nc"]
    in_maps = shard_inputs(x, h0, c0, Hebb0, weight_h, weight_x, bias, alpha, eta)
    res = run_bass_kernel_spmd(nc, in_maps, list(range(NCORES)))
    return gather_outputs(res.results)


# revision 39
# speedup vs baseline: 1.0633x; 1.0633x over previous
"""Trainium2 Bass kernel for a plastic (Hebbian fast-weight) LSTM cell.

Shapes (hardcoded): B=32, D=512, H=1024, 8 NeuronCores.
Sharding: hidden-dim sharding. Core c owns the h-slice [c*128, (c+1)*128) of:
  - Hebb0/Hebb1 last axis, h1/c1 columns, weight_h/weight_x/bias gate-column
    slices, alpha columns.
x / h0 are replicated (tiny). This beats batch sharding because the 28MB of
replicated weights dominate per-core DMA traffic otherwise.

Math per core (hs = 128-wide h slice, transposed "T" = h-major layout):
  gatesT[c, b]  = sum_k Wh[k, c] h0[b, k] + sum_d Wx[d, c] x[b, d] + bias[c]
  plastic[b, h] = sum_i h0[b,i] alpha[i,h] Hebb0[b,i,h]  (PE contraction of
                  m = alpha*Hebb0 against h0 columns)
  tg = tanh(g_gate + plastic);  c1 = sig(f)*c0 + sig(i)*tg; h1 = sig(o)*tanh(c1)
  Hebb1 = clip(Hebb0 + eta * h0[:,j] * tg[:,k], -1, 1)
"""

import os
from contextlib import ExitStack

import numpy as np

# The Bass kernel executes through jax/PJRT on the axon-tunneled NeuronCores;
# make sure the axon platform is visible even if the caller pinned cpu.
_p = os.environ.get("JAX_PLATFORMS")
if not _p:
    os.environ["JAX_PLATFORMS"] = "axon,cpu"
elif "axon" not in _p:
    os.environ["JAX_PLATFORMS"] = "axon," + _p

import concourse.bacc as bacc
import concourse.bass as bass
import concourse.mybir as mybir
import concourse.tile as tile
from concourse import dve_ops as _dvo
from concourse.bass_utils import run_bass_kernel_spmd
from concourse.dve_spec import (
    C0 as _C0,
    One as _One,
    Spec as _Spec,
    Src0 as _Src0,
    Src1 as _Src1,
    Zero as _Zero,
    lower as _lower,
    maxx as _maxx,
    minn as _minn,
)
from concourse.dve_uop import DveOpSpec as _DveOpSpec


def _get_fused_update_op():
    """Fused DVE op: out = clip(in0 * s0 + in1, -1, 1).

    One pass produces the updated Hebbian trace chunk from the tanh-gate
    broadcast (in0), per-partition eta*h0 column (s0), and Hebb0 (in1) —
    replacing scalar_tensor_tensor + a separate dual-scalar clip."""
    name = "PLASTIC_UPD_ANT"
    for op in _dvo.OPS:
        if op.name == name:
            return op
    spec = _Spec(
        body=_minn(_maxx(_Src0 * _C0 + _Src1, _Zero - _One), _One),
        reference=lambda in0, in1, c0, c1, c2: np.clip(in0 * c0 + in1, -1.0, 1.0),
    )
    opcode = max(_dvo._SUB_OPCODE_FOR_NAME.values()) + 1
    shas = {}
    for ver in ("v3", "v4"):
        s = _DveOpSpec(name=name, opcode=opcode, uops=_lower(spec, ver=ver),
                       rd1_en=_dvo.has_src1(spec))
        shas[ver] = s.sha(ver)
    op = _dvo.DveOp(name, spec, subdim=False, uops_sha=shas)
    _dvo.OPS.append(op)
    _dvo._SUB_OPCODE_FOR_NAME[name] = opcode
    _dvo.CUSTOM_DVE_SPECS[name] = spec
    return op

B, D, H, NCORES = 32, 512, 1024, 8
HS = H // NCORES  # 128
P = 128
F32 = mybir.dt.float32
GRP = 8  # batch-group size for the tanh/transpose stage
ALU = mybir.AluOpType
AFT = mybir.ActivationFunctionType

KC_H = H // P  # 8 contraction chunks for weight_h / alpha / hebb rows
KC_X = D // P  # 4 contraction chunks for weight_x


BF16 = mybir.dt.bfloat16


def build_kernel(hb_bufs=20, upd_bufs=4, m_bufs=3, bf16_m=False):
    """Build the per-core Bass/Tile program. Same program runs SPMD on all
    8 cores with different data."""
    nc = bacc.Bacc("TRN2", target_bir_lowering=False, debug=False, num_devices=NCORES)

    ein = lambda name, shape: nc.dram_tensor(name, shape, F32, kind="ExternalInput").ap()
    eout = lambda name, shape: nc.dram_tensor(name, shape, F32, kind="ExternalOutput").ap()

    hebb = ein("hebb", [B, H, HS])
    wh = ein("wh", [H, 4 * HS])
    wx = ein("wx", [D, 4 * HS])
    biasd = ein("biasv", [1, 4 * HS])
    alphad = ein("alphav", [H, HS])
    h0t = ein("h0t", [H, B])
    xt = ein("xt", [D, B])
    c0n = ein("c0n", [B, HS])
    etad = ein("etav", [1, 1])
    identd = ein("ident", [P, P])

    h1n = eout("h1n", [B, HS])
    c1n = eout("c1n", [B, HS])
    hebb1 = eout("hebb1", [B, H, HS])

    # Row-major partition split: partition p holds rows 8p..8p+7, so each
    # partition's DMA segment is 4KB contiguous (128 descriptors per DMA).
    hebb_r = hebb.rearrange("b (p r) k -> b p r k", r=8)
    hebb1_r = hebb1.rearrange("b (p r) k -> b p r k", r=8)

    with tile.TileContext(nc) as tc, ExitStack() as ctx:
        const = ctx.enter_context(tc.tile_pool(name="const", bufs=1))
        hpool = ctx.enter_context(tc.tile_pool(name="hebbp", bufs=hb_bufs))
        mpool = ctx.enter_context(tc.tile_pool(name="mp", bufs=m_bufs))
        upool = ctx.enter_context(tc.tile_pool(name="up", bufs=upd_bufs))
        tgbp = ctx.enter_context(tc.tile_pool(name="tgbp", bufs=3))
        tgrp = ctx.enter_context(tc.tile_pool(name="tgrp", bufs=3))
        smallp = ctx.enter_context(tc.tile_pool(name="smallp", bufs=2))
        psA = ctx.enter_context(tc.tile_pool(name="psA", bufs=1, space="PSUM"))
        psB = ctx.enter_context(tc.tile_pool(name="psB", bufs=2, space="PSUM"))
        psC = ctx.enter_context(tc.tile_pool(name="psC", bufs=3, space="PSUM"))

        dma = nc.sync.dma_start  # HWDGE
        fused_upd = _get_fused_update_op()

        # ---- critical-path constants first: what the DVE pipeline needs ----
        alpha_sb = const.tile([P, KC_H * HS], F32)  # [128, 1024]
        dma(alpha_sb[:].rearrange("p (r k) -> p r k", r=8),
            alphad.rearrange("(p r) k -> p r k", r=8))
        h0t2_sb = const.tile([P, 8 * B], F32)  # [128, 256] i = 8p + r layout
        dma(h0t2_sb[:].rearrange("p (r b) -> p r b", r=8),
            h0t.rearrange("(p r) b -> p r b", r=8))
        eta_sb = const.tile([1, 1], F32)
        dma(eta_sb[:], etad)
        ident_sb = const.tile([P, P], F32)
        dma(ident_sb[:], identd)

        # ---- remaining constants (before the Hebb stream: the gates — and
        # through them every per-b tanh chain — depend on the weights) ----
        wh_sb = const.tile([P, KC_H * 4 * HS], F32)  # [128, 4096]
        dma(wh_sb[:].rearrange("p (kc c) -> p kc c", kc=KC_H),
            wh.rearrange("(kc p) c -> p kc c", p=P))
        wx_sb = const.tile([P, KC_X * 4 * HS], F32)  # [128, 2048]
        dma(wx_sb[:].rearrange("p (dc c) -> p dc c", dc=KC_X),
            wx.rearrange("(dc p) c -> p dc c", p=P))
        h0t_sb = const.tile([P, KC_H * B], F32)  # [128, 256] gates layout
        dma(h0t_sb[:].rearrange("p (kc b) -> p kc b", kc=KC_H),
            h0t.rearrange("(kc p) b -> p kc b", p=P))
        xt_sb = const.tile([P, KC_X * B], F32)  # [128, 128]
        dma(xt_sb[:].rearrange("p (dc b) -> p dc b", dc=KC_X),
            xt.rearrange("(dc p) b -> p dc b", p=P))
        c0n_sb = const.tile([B, HS], F32)
        dma(c0n_sb[:], c0n)
        bias_sb = const.tile([1, 4 * HS], F32)
        dma(bias_sb[:], biasd)

        # Prefetch the first Hebb tiles behind the weights
        hbs = {}
        for b in range(GRP):
            hb = hpool.tile([P, KC_H * HS], F32, tag="hb")
            dma(hb[:].rearrange("p (r k) -> p r k", r=8), hebb_r[b])
            hbs[b] = hb

        ones_sb = const.tile([1, P], F32)
        nc.vector.memset(ones_sb[:], 1.0)

        # eta broadcast to all 128 partitions: ones_col @ eta
        eta_ps = psC.tile([P, 1], F32, tag="small")
        nc.tensor.matmul(eta_ps[:], ones_sb[:, 0:P], eta_sb[:], start=True, stop=True)
        eta128_sb = const.tile([P, 1], F32)
        nc.scalar.copy(eta128_sb[:], eta_ps[:])

        # eh0t = eta * h0T in the i = 8p + r layout (per-partition scale)
        eh0t_sb = const.tile([P, 8 * B], F32)
        nc.scalar.mul(eh0t_sb[:], h0t2_sb[:], eta128_sb[:, 0:1])

        mdt = BF16 if bf16_m else F32
        if bf16_m:
            h0t2_mm = const.tile([P, 8 * B], BF16)
            nc.vector.tensor_copy(h0t2_mm[:], h0t2_sb[:])
        else:
            h0t2_mm = h0t2_sb

        # ---- gate matmuls (transposed: [c, b]) ----
        pg_fio = psA.tile([P, 3 * B], F32)  # f|i|o chunks, one bank
        pg_g = psA.tile([P, B], F32)

        for q in range(4):
            out = pg_g[:] if q == 3 else pg_fio[:, q * B:(q + 1) * B]
            for kc in range(KC_H):
                nc.tensor.matmul(
                    out, wh_sb[:, kc * 512 + q * HS: kc * 512 + (q + 1) * HS],
                    h0t_sb[:, kc * B:(kc + 1) * B],
                    start=(kc == 0), stop=False)
            for dc in range(KC_X):
                nc.tensor.matmul(
                    out, wx_sb[:, dc * 512 + q * HS: dc * 512 + (q + 1) * HS],
                    xt_sb[:, dc * B:(dc + 1) * B],
                    start=False, stop=False)
            nc.tensor.matmul(out, bias_sb[:, q * HS:(q + 1) * HS],
                             ones_sb[:, 0:B], start=False, stop=True)

        sig_sb = const.tile([P, 3 * B], F32)
        nc.scalar.activation(sig_sb[:], pg_fio[:], AFT.Sigmoid)

        tgnat_sb = const.tile([B, HS], F32)  # tanh-gate rows, natural [b, k]

        for b in range(B):
            if b in hbs:
                hb = hbs.pop(b)
            else:
                hb = hpool.tile([P, KC_H * HS], F32, tag="hb")  # [128, 1024]
                dma(hb[:].rearrange("p (r k) -> p r k", r=8), hebb_r[b])
            m = mpool.tile([P, KC_H * HS], mdt, tag="m")
            nc.vector.tensor_mul(m[:], hb[:], alpha_sb[:])
            # plastic[b] + g-gate as one PSUM row: h0 column stationary (P=1
            # ldweights), m chunks moving; then the g-gate column transposed
            # in (also P=1 ldweights) to finish g_g[b] + plastic[b].
            pl_row = psC.tile([1, HS], F32, tag="small")
            for r in range(8):
                nc.tensor.matmul(
                    pl_row[:],
                    h0t2_mm[:, r * B + b: r * B + b + 1],
                    m[:, r * HS:(r + 1) * HS],
                    start=(r == 0), stop=False)
            gcol_sb = tgrp.tile([P, 1], F32, tag="gcol")
            nc.scalar.copy(gcol_sb[:], pg_g[:, b:b + 1])
            nc.tensor.matmul(pl_row[:], gcol_sb[:], ident_sb[:],
                             start=False, stop=True, is_transpose=True)
            # tanh directly on the row -> tg[b] at partition 0
            tgrow = tgrp.tile([1, P], F32, tag="tgrow")
            nc.scalar.activation(tgrow[:], pl_row[:], AFT.Tanh)
            # stash the row for the c1/h1 tail (sb->sb DMA moves partitions)
            nc.scalar.dma_start(tgnat_sb[b:b + 1, :], tgrow[:])
            # broadcast row to 128 partitions
            tgb_ps = psB.tile([P, P], F32, tag="tgb")
            nc.tensor.matmul(tgb_ps[:], ones_sb[:, 0:P], tgrow[:],
                             start=True, stop=True)
            tgb_sb = tgbp.tile([P, P], F32, tag="tgbs")
            nc.scalar.copy(tgb_sb[:], tgb_ps[:])
            # upd = clip(Hebb0 + (eta*h0_col) * tg_bcast, -1, 1) fused
            upd = upool.tile([P, KC_H * HS], F32, tag="upd")
            for r in range(8):
                nc.vector._custom_dve(
                    fused_upd,
                    out=upd[:, r * HS:(r + 1) * HS],
                    in0=tgb_sb[:],
                    in1=hb[:, r * HS:(r + 1) * HS],
                    s0=eh0t_sb[:, r * B + b: r * B + b + 1])
            nc.scalar.dma_start(
                hebb1_r[b], upd[:].rearrange("p (r k) -> p r k", r=8))

        # ---- tail: c1, h1 in natural [b, k] orientation ----
        # PE transposes turn the sigmoid columns into natural rows (each at
        # partition base 0 — tensor ops need matching base partitions)
        signat = []
        for q in range(3):
            sp = psB.tile([B, P], F32, tag="tgb")
            nc.tensor.transpose(sp[:], sig_sb[:, q * B:(q + 1) * B], ident_sb[:])
            sn = const.tile([B, P], F32)
            nc.scalar.copy(sn[:], sp[:])
            signat.append(sn)
        tmp1 = const.tile([B, HS], F32)
        tmp2 = const.tile([B, HS], F32)
        nc.vector.tensor_mul(tmp1[:], signat[0][:], c0n_sb[:])
        nc.vector.tensor_mul(tmp2[:], signat[1][:], tgnat_sb[:])
        c1n_sb = const.tile([B, HS], F32)
        nc.vector.tensor_add(c1n_sb[:], tmp1[:], tmp2[:])
        dma(c1n, c1n_sb[:])
        th_sb = const.tile([B, HS], F32)
        nc.scalar.activation(th_sb[:], c1n_sb[:], AFT.Tanh)
        h1n_sb = const.tile([B, HS], F32)
        nc.vector.tensor_mul(h1n_sb[:], signat[2][:], th_sb[:])
        dma(h1n, h1n_sb[:])

    nc.compile()
    return nc


def shard_inputs(x, h0, c0, Hebb0, weight_h, weight_x, bias, alpha, eta):
    """Host-side layout prep + hidden-dim sharding -> per-core input maps."""
    f32 = np.float32
    h0t = np.ascontiguousarray(h0.T, dtype=f32)
    xt = np.ascontiguousarray(x.T, dtype=f32)
    ident = np.eye(P, dtype=f32)
    eta2 = np.asarray(eta, dtype=f32).reshape(1, 1)
    in_maps = []
    for c in range(NCORES):
        sl = slice(c * HS, (c + 1) * HS)
        wh_c = np.ascontiguousarray(
            np.concatenate([weight_h[:, g * H:][:, sl] for g in range(4)], axis=1),
            dtype=f32)
        wx_c = np.ascontiguousarray(
            np.concatenate([weight_x[:, g * H:][:, sl] for g in range(4)], axis=1),
            dtype=f32)
        bias_c = np.ascontiguousarray(
            np.concatenate([bias[g * H:][sl] for g in range(4)])[None, :], dtype=f32)
        in_maps.append({
            "hebb": np.ascontiguousarray(Hebb0[:, :, sl], dtype=f32),
            "wh": wh_c,
            "wx": wx_c,
            "biasv": bias_c,
            "alphav": np.ascontiguousarray(alpha[:, sl], dtype=f32),
            "h0t": h0t,
            "xt": xt,
            "c0n": np.ascontiguousarray(c0[:, sl], dtype=f32),
            "etav": eta2,
            "ident": ident,
        })
    return in_maps


def gather_outputs(results):
    h1 = np.empty((B, H), dtype=np.float32)
    c1 = np.empty((B, H), dtype=np.float32)
    Hebb1 = np.empty((B, H, H), dtype=np.float32)
    for c in range(NCORES):
        sl = slice(c * HS, (c + 1) * HS)
        r = results[c]
        h1[:, sl] = r["h1n"]
        c1[:, sl] = r["c1n"]
        Hebb1[:, :, sl] = r["hebb1"]
    return h1, c1, Hebb1


_NC_CACHE = {}


def kernel(x, h0, c0, Hebb0, weight_h, weight_x, bias, alpha, eta):
    if "nc" not in _NC_CACHE:
        _NC_CACHE["nc"] = build_kernel()
    nc = _NC_CACHE["nc"]
    in_maps = shard_inputs(x, h0, c0, Hebb0, weight_h, weight_x, bias, alpha, eta)
    res = run_bass_kernel_spmd(nc, in_maps, list(range(NCORES)))
    return gather_outputs(res.results)


# revision 43
# speedup vs baseline: 1.0848x; 1.0202x over previous
"""Trainium2 Bass kernel for a plastic (Hebbian fast-weight) LSTM cell.

Shapes (hardcoded): B=32, D=512, H=1024, 8 NeuronCores.
Sharding: hidden-dim sharding. Core c owns the h-slice [c*128, (c+1)*128) of:
  - Hebb0/Hebb1 last axis, h1/c1 columns, weight_h/weight_x/bias gate-column
    slices, alpha columns.
x / h0 are replicated (tiny). This beats batch sharding because the 28MB of
replicated weights dominate per-core DMA traffic otherwise.

Math per core (hs = 128-wide h slice, transposed "T" = h-major layout):
  gatesT[c, b]  = sum_k Wh[k, c] h0[b, k] + sum_d Wx[d, c] x[b, d] + bias[c]
  plastic[b, h] = sum_i h0[b,i] alpha[i,h] Hebb0[b,i,h]  (PE contraction of
                  m = alpha*Hebb0 against h0 columns)
  tg = tanh(g_gate + plastic);  c1 = sig(f)*c0 + sig(i)*tg; h1 = sig(o)*tanh(c1)
  Hebb1 = clip(Hebb0 + eta * h0[:,j] * tg[:,k], -1, 1)
"""

import os
from contextlib import ExitStack

import numpy as np

# The Bass kernel executes through jax/PJRT on the axon-tunneled NeuronCores;
# make sure the axon platform is visible even if the caller pinned cpu.
_p = os.environ.get("JAX_PLATFORMS")
if not _p:
    os.environ["JAX_PLATFORMS"] = "axon,cpu"
elif "axon" not in _p:
    os.environ["JAX_PLATFORMS"] = "axon," + _p

import concourse.bacc as bacc
import concourse.bass as bass
import concourse.mybir as mybir
import concourse.tile as tile
from concourse import dve_ops as _dvo
from concourse.bass_utils import run_bass_kernel_spmd
from concourse.dve_spec import (
    C0 as _C0,
    One as _One,
    Spec as _Spec,
    Src0 as _Src0,
    Src1 as _Src1,
    Zero as _Zero,
    lower as _lower,
    maxx as _maxx,
    minn as _minn,
)
from concourse.dve_uop import DveOpSpec as _DveOpSpec


def _get_fused_update_op():
    """Fused DVE op: out = clip(in0 * s0 + in1, -1, 1).

    One pass produces the updated Hebbian trace chunk from the tanh-gate
    broadcast (in0), per-partition eta*h0 column (s0), and Hebb0 (in1) —
    replacing scalar_tensor_tensor + a separate dual-scalar clip."""
    name = "PLASTIC_UPD_ANT"
    for op in _dvo.OPS:
        if op.name == name:
            return op
    spec = _Spec(
        body=_minn(_maxx(_Src0 * _C0 + _Src1, _Zero - _One), _One),
        reference=lambda in0, in1, c0, c1, c2: np.clip(in0 * c0 + in1, -1.0, 1.0),
    )
    opcode = max(_dvo._SUB_OPCODE_FOR_NAME.values()) + 1
    shas = {}
    for ver in ("v3", "v4"):
        s = _DveOpSpec(name=name, opcode=opcode, uops=_lower(spec, ver=ver),
                       rd1_en=_dvo.has_src1(spec))
        shas[ver] = s.sha(ver)
    op = _dvo.DveOp(name, spec, subdim=False, uops_sha=shas)
    _dvo.OPS.append(op)
    _dvo._SUB_OPCODE_FOR_NAME[name] = opcode
    _dvo.CUSTOM_DVE_SPECS[name] = spec
    return op

B, D, H, NCORES = 32, 512, 1024, 8
HS = H // NCORES  # 128
P = 128
F32 = mybir.dt.float32
GRP = 8  # batch-group size for the tanh/transpose stage
ALU = mybir.AluOpType
AFT = mybir.ActivationFunctionType

KC_H = H // P  # 8 contraction chunks for weight_h / alpha / hebb rows
KC_X = D // P  # 4 contraction chunks for weight_x


BF16 = mybir.dt.bfloat16


def build_kernel(hb_bufs=20, upd_bufs=4, m_bufs=8, bf16_m=False):
    """Build the per-core Bass/Tile program. Same program runs SPMD on all
    8 cores with different data."""
    nc = bacc.Bacc("TRN2", target_bir_lowering=False, debug=False, num_devices=NCORES)

    ein = lambda name, shape: nc.dram_tensor(name, shape, F32, kind="ExternalInput").ap()
    eout = lambda name, shape: nc.dram_tensor(name, shape, F32, kind="ExternalOutput").ap()

    hebb = ein("hebb", [B, H, HS])
    wh = ein("wh", [H, 4 * HS])
    wx = ein("wx", [D, 4 * HS])
    biasd = ein("biasv", [1, 4 * HS])
    alphad = ein("alphav", [H, HS])
    h0t = ein("h0t", [H, B])
    xt = ein("xt", [D, B])
    c0n = ein("c0n", [B, HS])
    etad = ein("etav", [1, 1])
    identd = ein("ident", [P, P])

    h1n = eout("h1n", [B, HS])
    c1n = eout("c1n", [B, HS])
    hebb1 = eout("hebb1", [B, H, HS])

    # Row-major partition split: partition p holds rows 8p..8p+7, so each
    # partition's DMA segment is 4KB contiguous (128 descriptors per DMA).
    hebb_r = hebb.rearrange("b (p r) k -> b p r k", r=8)
    hebb1_r = hebb1.rearrange("b (p r) k -> b p r k", r=8)

    with tile.TileContext(nc) as tc, ExitStack() as ctx:
        const = ctx.enter_context(tc.tile_pool(name="const", bufs=1))
        hpool = ctx.enter_context(tc.tile_pool(name="hebbp", bufs=hb_bufs))
        mpool = ctx.enter_context(tc.tile_pool(name="mp", bufs=m_bufs))
        upool = ctx.enter_context(tc.tile_pool(name="up", bufs=upd_bufs))
        tgbp = ctx.enter_context(tc.tile_pool(name="tgbp", bufs=3))
        tgrp = ctx.enter_context(tc.tile_pool(name="tgrp", bufs=3))
        smallp = ctx.enter_context(tc.tile_pool(name="smallp", bufs=2))
        psA = ctx.enter_context(tc.tile_pool(name="psA", bufs=1, space="PSUM"))
        psB = ctx.enter_context(tc.tile_pool(name="psB", bufs=2, space="PSUM"))
        psC = ctx.enter_context(tc.tile_pool(name="psC", bufs=3, space="PSUM"))

        dma = nc.sync.dma_start  # HWDGE
        fused_upd = _get_fused_update_op()

        # ---- critical-path constants first: what the DVE pipeline needs ----
        alpha_sb = const.tile([P, KC_H * HS], F32)  # [128, 1024]
        dma(alpha_sb[:].rearrange("p (r k) -> p r k", r=8),
            alphad.rearrange("(p r) k -> p r k", r=8))
        h0t2_sb = const.tile([P, 8 * B], F32)  # [128, 256] i = 8p + r layout
        dma(h0t2_sb[:].rearrange("p (r b) -> p r b", r=8),
            h0t.rearrange("(p r) b -> p r b", r=8))
        eta_sb = const.tile([1, 1], F32)
        dma(eta_sb[:], etad)
        ident_sb = const.tile([P, P], F32)
        dma(ident_sb[:], identd)

        # ---- remaining constants (before the Hebb stream: the gates — and
        # through them every per-b tanh chain — depend on the weights).
        # Chunked per contraction block so gate matmuls overlap the transfer.
        wh_sb = const.tile([P, KC_H * 4 * HS], F32)  # [128, 4096]
        wh_r = wh.rearrange("(kc p) c -> kc p c", p=P)
        for kc in range(KC_H):
            dma(wh_sb[:, kc * 512:(kc + 1) * 512], wh_r[kc])
        wx_sb = const.tile([P, KC_X * 4 * HS], F32)  # [128, 2048]
        wx_r = wx.rearrange("(dc p) c -> dc p c", p=P)
        for dc in range(KC_X):
            dma(wx_sb[:, dc * 512:(dc + 1) * 512], wx_r[dc])
        h0t_sb = const.tile([P, KC_H * B], F32)  # [128, 256] gates layout
        dma(h0t_sb[:].rearrange("p (kc b) -> p kc b", kc=KC_H),
            h0t.rearrange("(kc p) b -> p kc b", p=P))
        xt_sb = const.tile([P, KC_X * B], F32)  # [128, 128]
        dma(xt_sb[:].rearrange("p (dc b) -> p dc b", dc=KC_X),
            xt.rearrange("(dc p) b -> p dc b", p=P))
        c0n_sb = const.tile([B, HS], F32)
        dma(c0n_sb[:], c0n)
        bias_sb = const.tile([1, 4 * HS], F32)
        dma(bias_sb[:], biasd)

        # Prefetch the first Hebb tiles behind the weights
        hbs = {}
        for b in range(GRP):
            hb = hpool.tile([P, KC_H * HS], F32, tag="hb")
            dma(hb[:].rearrange("p (r k) -> p r k", r=8), hebb_r[b])
            hbs[b] = hb

        ones_sb = const.tile([1, P], F32)
        nc.vector.memset(ones_sb[:], 1.0)

        # eta broadcast to all 128 partitions: ones_col @ eta
        eta_ps = psC.tile([P, 1], F32, tag="small")
        nc.tensor.matmul(eta_ps[:], ones_sb[:, 0:P], eta_sb[:], start=True, stop=True)
        eta128_sb = const.tile([P, 1], F32)
        nc.scalar.copy(eta128_sb[:], eta_ps[:])

        # eh0t = eta * h0T in the i = 8p + r layout (per-partition scale)
        eh0t_sb = const.tile([P, 8 * B], F32)
        nc.scalar.mul(eh0t_sb[:], h0t2_sb[:], eta128_sb[:, 0:1])

        mdt = BF16 if bf16_m else F32
        if bf16_m:
            h0t2_mm = const.tile([P, 8 * B], BF16)
            nc.vector.tensor_copy(h0t2_mm[:], h0t2_sb[:])
        else:
            h0t2_mm = h0t2_sb

        # ---- gate matmuls (transposed: [c, b]) ----
        pg_fio = psA.tile([P, 3 * B], F32)  # f|i|o chunks, one bank
        pg_g = psA.tile([P, B], F32)

        for q in range(4):
            out = pg_g[:] if q == 3 else pg_fio[:, q * B:(q + 1) * B]
            for kc in range(KC_H):
                nc.tensor.matmul(
                    out, wh_sb[:, kc * 512 + q * HS: kc * 512 + (q + 1) * HS],
                    h0t_sb[:, kc * B:(kc + 1) * B],
                    start=(kc == 0), stop=False)
            for dc in range(KC_X):
                nc.tensor.matmul(
                    out, wx_sb[:, dc * 512 + q * HS: dc * 512 + (q + 1) * HS],
                    xt_sb[:, dc * B:(dc + 1) * B],
                    start=False, stop=False)
            nc.tensor.matmul(out, bias_sb[:, q * HS:(q + 1) * HS],
                             ones_sb[:, 0:B], start=False, stop=True)

        sig_sb = const.tile([P, 3 * B], F32)
        nc.scalar.activation(sig_sb[:], pg_fio[:], AFT.Sigmoid)

        tgnat_sb = const.tile([B, HS], F32)  # tanh-gate rows, natural [b, k]

        for b in range(B):
            if b in hbs:
                hb = hbs.pop(b)
            else:
                hb = hpool.tile([P, KC_H * HS], F32, tag="hb")  # [128, 1024]
                dma(hb[:].rearrange("p (r k) -> p r k", r=8), hebb_r[b])
            m = mpool.tile([P, KC_H * HS], mdt, tag="m")
            nc.vector.tensor_mul(m[:], hb[:], alpha_sb[:])
            # plastic[b] + g-gate as one PSUM row: h0 column stationary (P=1
            # ldweights), m chunks moving; then the g-gate column transposed
            # in (also P=1 ldweights) to finish g_g[b] + plastic[b].
            pl_row = psC.tile([1, HS], F32, tag="small")
            for r in range(8):
                nc.tensor.matmul(
                    pl_row[:],
                    h0t2_mm[:, r * B + b: r * B + b + 1],
                    m[:, r * HS:(r + 1) * HS],
                    start=(r == 0), stop=False)
            gcol_sb = tgrp.tile([P, 1], F32, tag="gcol")
            nc.scalar.copy(gcol_sb[:], pg_g[:, b:b + 1])
            nc.tensor.matmul(pl_row[:], gcol_sb[:], ident_sb[:],
                             start=False, stop=True, is_transpose=True)
            # tanh directly on the row -> tg[b] at partition 0
            tgrow = tgrp.tile([1, P], F32, tag="tgrow")
            nc.scalar.activation(tgrow[:], pl_row[:], AFT.Tanh)
            # stash the row for the c1/h1 tail (sb->sb DMA moves partitions)
            nc.scalar.dma_start(tgnat_sb[b:b + 1, :], tgrow[:])
            # broadcast row to 128 partitions
            tgb_ps = psB.tile([P, P], F32, tag="tgb")
            nc.tensor.matmul(tgb_ps[:], ones_sb[:, 0:P], tgrow[:],
                             start=True, stop=True)
            tgb_sb = tgbp.tile([P, P], F32, tag="tgbs")
            nc.scalar.copy(tgb_sb[:], tgb_ps[:])
            # upd = clip(Hebb0 + (eta*h0_col) * tg_bcast, -1, 1) fused
            upd = upool.tile([P, KC_H * HS], F32, tag="upd")
            for r in range(8):
                nc.vector._custom_dve(
                    fused_upd,
                    out=upd[:, r * HS:(r + 1) * HS],
                    in0=tgb_sb[:],
                    in1=hb[:, r * HS:(r + 1) * HS],
                    s0=eh0t_sb[:, r * B + b: r * B + b + 1])
            nc.scalar.dma_start(
                hebb1_r[b], upd[:].rearrange("p (r k) -> p r k", r=8))

        # ---- tail: c1, h1 in natural [b, k] orientation ----
        # PE transposes turn the sigmoid columns into natural rows (each at
        # partition base 0 — tensor ops need matching base partitions)
        signat = []
        for q in range(3):
            sp = psB.tile([B, P], F32, tag="tgb")
            nc.tensor.transpose(sp[:], sig_sb[:, q * B:(q + 1) * B], ident_sb[:])
            sn = const.tile([B, P], F32)
            nc.scalar.copy(sn[:], sp[:])
            signat.append(sn)
        tmp1 = const.tile([B, HS], F32)
        tmp2 = const.tile([B, HS], F32)
        nc.vector.tensor_mul(tmp1[:], signat[0][:], c0n_sb[:])
        nc.vector.tensor_mul(tmp2[:], signat[1][:], tgnat_sb[:])
        c1n_sb = const.tile([B, HS], F32)
        nc.vector.tensor_add(c1n_sb[:], tmp1[:], tmp2[:])
        dma(c1n, c1n_sb[:])
        th_sb = const.tile([B, HS], F32)
        nc.scalar.activation(th_sb[:], c1n_sb[:], AFT.Tanh)
        h1n_sb = const.tile([B, HS], F32)
        nc.vector.tensor_mul(h1n_sb[:], signat[2][:], th_sb[:])
        dma(h1n, h1n_sb[:])

    nc.compile()
    return nc


def shard_inputs(x, h0, c0, Hebb0, weight_h, weight_x, bias, alpha, eta):
    """Host-side layout prep + hidden-dim sharding -> per-core input maps."""
    f32 = np.float32
    h0t = np.ascontiguousarray(h0.T, dtype=f32)
    xt = np.ascontiguousarray(x.T, dtype=f32)
    ident = np.eye(P, dtype=f32)
    eta2 = np.asarray(eta, dtype=f32).reshape(1, 1)
    in_maps = []
    for c in range(NCORES):
        sl = slice(c * HS, (c + 1) * HS)
        wh_c = np.ascontiguousarray(
            np.concatenate([weight_h[:, g * H:][:, sl] for g in range(4)], axis=1),
            dtype=f32)
        wx_c = np.ascontiguousarray(
            np.concatenate([weight_x[:, g * H:][:, sl] for g in range(4)], axis=1),
            dtype=f32)
        bias_c = np.ascontiguousarray(
            np.concatenate([bias[g * H:][sl] for g in range(4)])[None, :], dtype=f32)
        in_maps.append({
            "hebb": np.ascontiguousarray(Hebb0[:, :, sl], dtype=f32),
            "wh": wh_c,
            "wx": wx_c,
            "biasv": bias_c,
            "alphav": np.ascontiguousarray(alpha[:, sl], dtype=f32),
            "h0t": h0t,
            "xt": xt,
            "c0n": np.ascontiguousarray(c0[:, sl], dtype=f32),
            "etav": eta2,
            "ident": ident,
        })
    return in_maps


def gather_outputs(results):
    h1 = np.empty((B, H), dtype=np.float32)
    c1 = np.empty((B, H), dtype=np.float32)
    Hebb1 = np.empty((B, H, H), dtype=np.float32)
    for c in range(NCORES):
        sl = slice(c * HS, (c + 1) * HS)
        r = results[c]
        h1[:, sl] = r["h1n"]
        c1[:, sl] = r["c1n"]
        Hebb1[:, :, sl] = r["hebb1"]
    return h1, c1, Hebb1


_NC_CACHE = {}


def kernel(x, h0, c0, Hebb0, weight_h, weight_x, bias, alpha, eta):
    if "nc" not in _NC_CACHE:
        _NC_CACHE["nc"] = build_kernel()
    nc = _NC_CACHE["nc"]
    in_maps = shard_inputs(x, h0, c0, Hebb0, weight_h, weight_x, bias, alpha, eta)
    res = run_bass_kernel_spmd(nc, in_maps, list(range(NCORES)))
    return gather_outputs(res.results)
